# revision 21
# baseline (speedup 1.0000x reference)
"""Causal multi-head attention (B=2, S=2048, D=1024, H=16) on 8 trn2 cores.

Sharding: batch (2-way) x head-group (4-way) = 8 cores. Each core computes
QKV projection for its batch restricted to its 4 heads, causal attention,
and a row-parallel slice of the output projection; the host sums the 4
partial fp16 outputs per batch (the all-reduce of the row-parallel Wo
matmul) and adds bo.

Per-core kernel (Tile framework):
  - QK projection runs in fp8e4 with DoubleRow perf mode (host ships
    x and Wqk pair-packed as [128, 2, .] operands, Wqk pre-scaled by 256;
    the 1/256 rescale folds into the bias-add): 2x PE throughput, and the
    ~1e-4-scale logit error is negligible against the softmax.
  - V projection stays fp16 (output-accuracy critical).
  - Q,K live in [feat, seq] qkT tiles; a head pair occupies partitions
    0-63 / 64-127, so score matmuls (K=64) row-pack via tile_position
    (0,.) / (64,.) and run concurrently.
  - PV is col-packed: head-even -> PSUM rows 0-63, head-odd -> rows
    64-127 (M=64 col tiles), one [128,512] bank per pair; the output
    layout doubles as the Wo lhsT.
  - Softmax denominators accumulate in-sweep: after each base's four
    exps, 4-way col-tiled M=1 matmuls (ones.T @ es) add into psd
    partitions 0/32/64/96.
  - exp on ScalarE: one ACT per (head, kb-pair) over [128, <=1024] PSUM;
    causal masking in place on fp16 es via GpSimd affine_select.
  - Schedule: psn tiles rotate through a dedicated 2-slot PSUM ring;
    projection-chunk / Wo-output groups pop as fillers into their own
    1-bank slot; the per-qi normalization chain (reciprocal on DVE,
    K=1 broadcast matmuls, vw multiply) is split across the qi boundary
    and base-0 PV matmuls are deferred so the in-order PE stream never
    stalls on it.
"""

import numpy as np
import ml_dtypes
from contextlib import ExitStack

import concourse.bass as bass
import concourse.mybir as mybir
import concourse.tile as tile
from concourse import bacc
from concourse.bass_utils import run_bass_kernel_spmd

B, S, D, H, HD = 2, 2048, 1024, 16, 64
NCORES = 8
NHG = 4                  # head groups (cores per batch)
NH = H // NHG            # 4 local heads
FQK = NH * HD * 2        # 512 local q+k features
FV = NH * HD             # 256 local v features
QB = 512                 # query block (attention outer tile)
KB = 128                 # key block
NSC = S // QB            # 4 seq chunks
W8SCALE = 256.0          # fp8 Wqk pre-scale (undone in the bias add)
R32 = mybir.dt.float32r
F16 = mybir.dt.float16
F32 = mybir.dt.float32
F8 = mybir.dt.float8e4
EXP = mybir.ActivationFunctionType.Exp
SCALE = 1.0 / np.sqrt(HD)
DR = mybir.MatmulPerfMode.DoubleRow


def _build_body(ctx, tc, x_d, x8_d, wqk8_d, wv_d, bqk_d, bvb_d, wo_d, out_d):
    nc = tc.nc

    const = ctx.enter_context(tc.tile_pool(name="const", bufs=1))
    w8_pool = ctx.enter_context(tc.tile_pool(name="w8p", bufs=1))
    wvp = ctx.enter_context(tc.tile_pool(name="wvp", bufs=1))
    wop = ctx.enter_context(tc.tile_pool(name="wop", bufs=1))
    xt_pool = ctx.enter_context(tc.tile_pool(name="xtp", bufs=3))
    x8_pool = ctx.enter_context(tc.tile_pool(name="x8p", bufs=3))
    qk_pool = ctx.enter_context(tc.tile_pool(name="qkp", bufs=1))
    v_pool = ctx.enter_context(tc.tile_pool(name="vp", bufs=16))
    es_pool = ctx.enter_context(tc.tile_pool(name="ep", bufs=16))
    vw_pool = ctx.enter_context(tc.tile_pool(name="vwp", bufs=6))
    rc_pool = ctx.enter_context(tc.tile_pool(name="rcp", bufs=2))
    os_pool = ctx.enter_context(tc.tile_pool(name="osp", bufs=6))
    # PSUM budget (8 banks): psn ring 2x[128,1024]=4, psd 1, pw 1, poh 2
    stage = ctx.enter_context(tc.tile_pool(name="stage", bufs=2, space="PSUM"))
    pvp = ctx.enter_context(tc.tile_pool(name="pvp", bufs=2, space="PSUM"))

    # ---- constants ----
    ones16 = const.tile([128, 128], F16)
    nc.gpsimd.memset(ones16, 1.0)

    # ---- weights (first matmul needs only wqk8 + x80: issue them first) ----
    wqk8_sb = w8_pool.tile([128, 4, 2, FQK], F8, name="wqk8", tag="wqk8")
    for i in range(2):
        nc.sync.dma_start(wqk8_sb[:, :, i, :], wqk8_d.ap().rearrange(
            "(dcp p) (two f) -> p dcp two f", p=128, two=2)[:, :, i, :])
    x80 = x8_pool.tile([128, 4, 2, QB], F8, name="x8", tag="x8")
    for i in range(2):
        nc.sync.dma_start(x80[:, :, i, :], x8_d.ap().rearrange(
            "(dcp p) (two s) -> p dcp two s", p=128, two=2)[:, :, i, 0:QB])
    bqk_sb = const.tile([128, 4], F32)
    nc.sync.dma_start(bqk_sb, bqk_d.ap().rearrange("(f p) -> p f", p=128))
    bvb_sb = const.tile([128, FV], F32)
    nc.sync.dma_start(bvb_sb, bvb_d.ap())
    xT0 = xt_pool.tile([128, 8, QB], F16, name="xt", tag="xt")
    nc.sync.dma_start(xT0, x_d.ap().rearrange(
        "(dc p) s -> p dc s", p=128)[:, :, 0:QB])
    wv_sb = wvp.tile([128, 8, FV], F16, name="wv", tag="wv")
    nc.sync.dma_start(wv_sb, wv_d.ap().rearrange("(dc p) f -> p dc f", p=128))
    wo_sb = wop.tile([128, 2, D], F16, name="wo", tag="wo")
    nc.sync.dma_start(wo_sb, wo_d.ap().rearrange("(c p) f -> p c f", p=128))
    # ---- phase B: QKV projection groups ----
    qkT = [qk_pool.tile([128, S], F8, name=f"qkT{f}", tag=f"qkT{f}", bufs=1)
           for f in range(4)]
    v_tiles = [None] * (NSC * 4)

    def b_groups(sc, tag):
        """Closures for chunk sc: 4 fp8 QK feature groups + 4 fp16 V seq
        groups, each through one `tag` psum slot."""
        tiles = {}

        def load_x():
            if sc == 0:
                tiles["xT"], tiles["x8"] = xT0, x80
                return
            x8 = x8_pool.tile([128, 4, 2, QB], F8, name="x8", tag="x8")
            for i in range(2):
                nc.sync.dma_start(x8[:, :, i, :], x8_d.ap().rearrange(
                    "(dcp p) (two s) -> p dcp two s", p=128, two=2)
                    [:, :, i, sc * QB:(sc + 1) * QB])
            xt = xt_pool.tile([128, 8, QB], F16, name="xt", tag="xt")
            nc.sync.dma_start(xt, x_d.ap().rearrange(
                "(dc p) s -> p dc s", p=128)[:, :, sc * QB:(sc + 1) * QB])
            tiles["xT"], tiles["x8"] = xt, x8

        def qk_group(f):
            def emit():
                pq = stage.tile([128, QB], F32, name="pq", tag=tag,
                                bufs=(2 if tag == "stage" else 1))
                for dcp in range(4):
                    nc.tensor.matmul(
                        pq, wqk8_sb[:, dcp, :, f * 128:(f + 1) * 128],
                        tiles["x8"][:, dcp, :, :], start=(dcp == 0),
                        stop=(dcp == 3), perf_mode=DR)
                nc.vector.tensor_scalar(
                    qkT[f][:, sc * QB:(sc + 1) * QB], pq,
                    1.0 / W8SCALE, bqk_sb[:, f:f + 1],
                    op0=mybir.AluOpType.mult, op1=mybir.AluOpType.add)
            return emit

        def v_group(sb):
            def emit():
                pv = stage.tile([128, FV], F32, name="pvq", tag=tag,
                                bufs=(2 if tag == "stage" else 1))
                for dc in range(8):
                    nc.tensor.matmul(
                        pv, tiles["xT"][:, dc, sb * 128:(sb + 1) * 128],
                        wv_sb[:, dc, :], start=(dc == 0), stop=(dc == 7))
                vt = v_pool.tile([128, NH, HD], F16, name="vt", tag="vt")
                nc.vector.tensor_add(
                    vt, pv.rearrange("p (h e) -> p h e", h=NH),
                    bvb_sb.rearrange("p (h e) -> p h e", h=NH))
                v_tiles[sc * 4 + sb] = vt
            return emit

        def first():
            load_x()
            qk_group(0)()
        qk_groups = [first] + [qk_group(f) for f in range(1, 4)]
        return qk_groups, [v_group(sb) for sb in range(4)]

    # ---- phase C ----
    def koff(qi, kb):
        return max(0, (kb - qi * 4)) * KB

    filler = []

    def pop_filler(n=1):
        for _ in range(n):
            if filler:
                filler.pop(0)()

    def emit_unit(qi, base, h, poh, nkb, do_pv=True):
        """Scores + exp (+ PV) for one (head, kb-pair). Returns es."""
        p, e = h // 2, h % 2
        r0 = e * 64
        off0 = koff(qi, base)
        psn = stage.tile([128, 2 * QB], F32, name="psn", tag="stage")
        for j2 in range(2):
            kb = base + j2
            off = koff(qi, kb)
            nc.tensor.matmul(
                psn[:, j2 * QB + off:(j2 + 1) * QB],
                qkT[2 + p][r0:r0 + 64, kb * KB:(kb + 1) * KB],
                qkT[p][r0:r0 + 64, qi * QB + off:(qi + 1) * QB],
                start=True, stop=True, skip_group_check=True)
        es = es_pool.tile([128, 2 * QB], F16, name="es", tag="es")
        nc.scalar.activation(es[:, off0:], psn[:, off0:], EXP, scale=SCALE)
        for j2 in range(2):
            kb = base + j2
            j = kb - qi * 4
            if j >= 0:
                off = j * KB
                nc.gpsimd.affine_select(
                    out=es[:, j2 * QB + off:(j2 + 1) * QB],
                    in_=es[:, j2 * QB + off:(j2 + 1) * QB],
                    compare_op=mybir.AluOpType.is_ge,
                    fill=0.0, base=0,
                    pattern=[[1, QB - off]],
                    channel_multiplier=-1,
                )
        if do_pv:
            emit_pv(qi, base, h, poh, nkb, es)
        return es

    def emit_pv(qi, base, h, poh, nkb, es):
        p, e = h // 2, h % 2
        r0 = e * 64
        for j2 in range(2):
            kb = base + j2
            off = koff(qi, kb)
            nc.tensor.matmul(
                poh[p][r0:r0 + 64, off:QB],
                v_tiles[kb][:, h, :],
                es[:, j2 * QB + off:(j2 + 1) * QB],
                start=(kb == 0), stop=(kb == nkb - 1),
                skip_group_check=True)

    def emit_denoms(qi, base, nkb, psd, es4):
        for j2 in range(2):
            kb = base + j2
            off = koff(qi, kb)
            for h in range(4):
                nc.tensor.matmul(
                    psd[32 * h:32 * h + 1, off:QB],
                    ones16[:, 0:1],
                    es4[h][:, j2 * QB + off:(j2 + 1) * QB],
                    start=(kb == 0), stop=(kb == nkb - 1),
                    tile_position=(0, 32 * h),
                    skip_group_check=True)

    def make_denorm_b(qi, rc16, poh):
        """Closure: broadcast 1/denom + normalize poh into vwT, then queue
        the Wo output groups."""
        def run():
            vwT = []
            for p in range(2):
                pb2 = stage.tile([128, 2 * QB], F32, name="pb2", tag="stage")
                for e in range(2):
                    h = 2 * p + e
                    nc.tensor.matmul(pb2[:, e * QB:(e + 1) * QB],
                                     ones16[32 * h:32 * h + 1, :],
                                     rc16[32 * h:32 * h + 1, :],
                                     start=True, stop=True,
                                     tile_position=(32 * h, 0),
                                     skip_group_check=True)
                bcs = rc_pool.tile([128, QB], F32, name="bcs", tag="bcs")
                for e in range(2):
                    r0 = e * 64
                    nc.vector.tensor_copy(
                        bcs[r0:r0 + 64, :],
                        pb2[r0:r0 + 64, e * QB:(e + 1) * QB])
                vw = vw_pool.tile([128, QB], F16, name=f"vwT{p}", tag="vw")
                nc.vector.tensor_mul(vw, poh[p], bcs)
                vwT.append(vw)
            last = qi == NSC - 1
            filler.extend(
                wo_group(qi, vwT, ql, do, tag=("stage" if last else "pw"))
                for ql in range(4) for do in range(2))
        return run

    def wo_group(qi, vwT, ql, do, tag="pw"):
        def emit():
            pw = stage.tile([128, QB], F32, name="pw", tag=tag,
                            bufs=(1 if tag != "stage" else 2))
            for c in range(2):
                nc.tensor.matmul(pw, vwT[c][:, ql * 128:(ql + 1) * 128],
                                 wo_sb[:, c, do * QB:(do + 1) * QB],
                                 start=(c == 0), stop=(c == 1))
            osb = os_pool.tile([128, QB], F16, name="osb", tag="osb")
            nc.vector.tensor_copy(osb, pw)
            nc.sync.dma_start(
                out_d.ap()[qi * QB + ql * 128: qi * QB + (ql + 1) * 128,
                           do * QB:(do + 1) * QB], osb)
        return emit

    def emit_C(qi, denorm_b_prev):
        """Full attention for query chunk qi. Returns this qi's denorm_b."""
        nkb = (qi + 1) * 4
        poh = None
        es0 = []
        psd = stage.tile([128, QB], F32, name="psd", tag="psd", bufs=1)
        for base in range(0, nkb, 2):
            es4 = []
            for h in range(4):
                es = emit_unit(qi, base, h, poh, nkb, do_pv=(base > 0))
                es4.append(es)
                if base > 0:
                    pop_filler()
            emit_denoms(qi, base, nkb, psd, es4)
            if base == 0:
                # qi-boundary catch-up while this qi's first exps drain:
                # previous qi's normalization (frees its poh slots), then
                # this qi's deferred base-0 PV matmuls.
                if denorm_b_prev is not None:
                    denorm_b_prev()
                pop_filler(2)
                poh = [pvp.tile([128, QB], F32, name=f"poh{p}", tag="pv")
                       for p in range(2)]
                for h in range(4):
                    emit_pv(qi, base, h, poh, nkb, es4[h])
        # reciprocal chain (DVE only; PE stream continues into next qi)
        dsb = rc_pool.tile([128, QB], F32, name="dsb", tag="dsb")
        nc.vector.tensor_copy(dsb, psd)
        rcf = rc_pool.tile([128, QB], F32, name="rcf", tag="rcf")
        nc.vector.reciprocal_approx_fast(rcf, dsb)
        rc16 = rc_pool.tile([128, QB], F16, name="rc16", tag="rc16")
        nc.vector.tensor_copy(rc16, rcf)
        return make_denorm_b(qi, rc16, poh)

    # ---- schedule ----
    qk0, v0 = b_groups(0, "stage")
    for g in qk0 + v0:
        g()
    denorm_b = None
    for qi in range(NSC):
        if qi in (0, 1):
            # B1 drains during C0, B2 during C1
            qk, vg = b_groups(qi + 1, "pw")
            filler.extend(qk + vg)
        elif qi == NSC - 1:
            # B3: C3's scores need chunk-3 Q/K up front, so the QK groups
            # emit directly (they fill C2's ScalarE-bound tail); the V
            # groups (only needed from kb block 12) drain as C3 fillers.
            qk, vg = b_groups(qi, "pw")
            for g in qk:
                g()
            filler.extend(vg)
        denorm_b = emit_C(qi, denorm_b)
    denorm_b()
    while filler:
        filler.pop(0)()


_COMPILED = None


def get_compiled():
    global _COMPILED
    if _COMPILED is not None:
        return _COMPILED
    nc = bacc.Bacc("TRN2", target_bir_lowering=False, debug=False,
                   enable_asserts=False, num_devices=NCORES)
    x_d = nc.dram_tensor("x", [D, S], F16, kind="ExternalInput")
    x8_d = nc.dram_tensor("x8", [D // 2, 2 * S], F8, kind="ExternalInput")
    wqk8_d = nc.dram_tensor("wqk8", [D // 2, 2 * FQK], F8,
                            kind="ExternalInput")
    wv_d = nc.dram_tensor("wv", [D, FV], F16, kind="ExternalInput")
    bqk_d = nc.dram_tensor("bqk", [FQK], F32, kind="ExternalInput")
    bvb_d = nc.dram_tensor("bvb", [128, FV], F32, kind="ExternalInput")
    wo_d = nc.dram_tensor("wo", [FV, D], F16, kind="ExternalInput")
    out_d = nc.dram_tensor("out", [S, D], F16, kind="ExternalOutput")
    with tile.TileContext(nc) as tc:
        with ExitStack() as ctx:
            _build_body(ctx, tc, x_d, x8_d, wqk8_d, wv_d, bqk_d, bvb_d, wo_d,
                        out_d)
    nc.compile()
    _COMPILED = nc
    return nc


def _pack_pairs(a):
    """[256*k, n] -> pair-packed [128*k, 2*n] for DoubleRow operands."""
    k = a.shape[0] // 256
    return np.ascontiguousarray(
        a.reshape(k, 2, 128, a.shape[1]).transpose(0, 2, 1, 3)
        .reshape(k * 128, 2 * a.shape[1]))


def make_in_maps(x, Wqkv, bqkv, Wo):
    f8 = ml_dtypes.float8_e4m3fn
    x = np.ascontiguousarray(np.asarray(x, dtype=np.float32))
    Wqkv = np.asarray(Wqkv, dtype=np.float32)
    bqkv = np.asarray(bqkv, dtype=np.float32)
    Wo = np.asarray(Wo, dtype=np.float32)
    in_maps = []
    for c in range(NCORES):
        b, hg = divmod(c, NHG)
        qs = slice(hg * FV, (hg + 1) * FV)
        ks = slice(D + hg * FV, D + (hg + 1) * FV)
        vs = slice(2 * D + hg * FV, 2 * D + (hg + 1) * FV)
        xT = np.ascontiguousarray(x[b].T)  # [D, S] f32
        wqk = np.concatenate([Wqkv[:, qs], Wqkv[:, ks]], axis=1)  # [D, FQK]
        in_maps.append({
            "x": xT.astype(np.float16),
            "x8": _pack_pairs(xT).astype(f8),
            "wqk8": _pack_pairs(wqk * W8SCALE).astype(f8),
            "wv": np.ascontiguousarray(Wqkv[:, vs]).astype(np.float16),
            "bqk": np.ascontiguousarray(
                np.concatenate([bqkv[qs], bqkv[ks]])),
            "bvb": np.ascontiguousarray(
                np.broadcast_to(bqkv[vs], (128, FV)).copy()),
            "wo": np.ascontiguousarray(Wo[hg * FV:(hg + 1) * FV, :]).astype(np.float16),
        })
    return in_maps


def run_sharded(x, Wqkv, bqkv, Wo, bo, **spmd_kwargs):
    nc = get_compiled()
    in_maps = make_in_maps(x, Wqkv, bqkv, Wo)
    res = run_bass_kernel_spmd(nc, in_maps, core_ids=list(range(NCORES)),
                               **spmd_kwargs)
    out = np.zeros((B, S, D), np.float32)
    for c in range(NCORES):
        out[c // NHG] += res.results[c]["out"].astype(np.float32)
    out += np.asarray(bo, dtype=np.float32)
    return out, res


def kernel(x, mask, Wqkv, bqkv, Wo, bo):
    out, _ = run_sharded(x, Wqkv, bqkv, Wo, bo)
    return out


# revision 23
# speedup vs baseline: 1.0244x; 1.0244x over previous
"""Causal multi-head attention (B=2, S=2048, D=1024, H=16) on 8 trn2 cores.

Sharding: batch (2-way) x head-group (4-way) = 8 cores. Each core computes
QKV projection for its batch restricted to its 4 heads, causal attention,
and a row-parallel slice of the output projection; the host sums the 4
partial fp16 outputs per batch (the all-reduce of the row-parallel Wo
matmul) and adds bo.

Per-core kernel (Tile framework):
  - QK projection runs in fp8e4 with DoubleRow perf mode (host ships
    x and Wqk pair-packed as [128, 2, .] operands, Wqk pre-scaled by 256;
    the 1/256 rescale folds into the bias-add): 2x PE throughput, and the
    ~1e-4-scale logit error is negligible against the softmax.
  - V projection stays fp16 (output-accuracy critical).
  - Q,K live in [feat, seq] qkT tiles; a head pair occupies partitions
    0-63 / 64-127, so score matmuls (K=64) row-pack via tile_position
    (0,.) / (64,.) and run concurrently.
  - PV is col-packed: head-even -> PSUM rows 0-63, head-odd -> rows
    64-127 (M=64 col tiles), one [128,512] bank per pair; the output
    layout doubles as the Wo lhsT.
  - Softmax denominators accumulate in-sweep: after each base's four
    exps, 4-way col-tiled M=1 matmuls (ones.T @ es) add into psd
    partitions 0/32/64/96.
  - exp on ScalarE: one ACT per (head, kb-pair) over [128, <=1024] PSUM;
    causal masking in place on fp16 es via GpSimd affine_select.
  - Schedule: psn tiles rotate through a dedicated 2-slot PSUM ring;
    projection-chunk / Wo-output groups pop as fillers into their own
    1-bank slot; the per-qi normalization chain (reciprocal on DVE,
    K=1 broadcast matmuls, vw multiply) is split across the qi boundary
    and base-0 PV matmuls are deferred so the in-order PE stream never
    stalls on it.
"""

import numpy as np
import ml_dtypes
from contextlib import ExitStack

import concourse.bass as bass
import concourse.mybir as mybir
import concourse.tile as tile
from concourse import bacc
from concourse.bass_utils import run_bass_kernel_spmd

B, S, D, H, HD = 2, 2048, 1024, 16, 64
NCORES = 8
NHG = 4                  # head groups (cores per batch)
NH = H // NHG            # 4 local heads
FQK = NH * HD * 2        # 512 local q+k features
FV = NH * HD             # 256 local v features
QB = 512                 # query block (attention outer tile)
KB = 128                 # key block
NSC = S // QB            # 4 seq chunks
W8SCALE = 256.0          # fp8 Wqk pre-scale (undone in the bias add)
R32 = mybir.dt.float32r
F16 = mybir.dt.float16
F32 = mybir.dt.float32
F8 = mybir.dt.float8e4
EXP = mybir.ActivationFunctionType.Exp
SCALE = 1.0 / np.sqrt(HD)
DR = mybir.MatmulPerfMode.DoubleRow


def _build_body(ctx, tc, x_d, x8_d, wqk8_d, wv_d, bqk_d, bvb_d, wo_d, out_d):
    nc = tc.nc

    const = ctx.enter_context(tc.tile_pool(name="const", bufs=1))
    w8_pool = ctx.enter_context(tc.tile_pool(name="w8p", bufs=1))
    wvp = ctx.enter_context(tc.tile_pool(name="wvp", bufs=1))
    wop = ctx.enter_context(tc.tile_pool(name="wop", bufs=1))
    xt_pool = ctx.enter_context(tc.tile_pool(name="xtp", bufs=2))
    x8_pool = ctx.enter_context(tc.tile_pool(name="x8p", bufs=2))
    qk_pool = ctx.enter_context(tc.tile_pool(name="qkp", bufs=1))
    v_pool = ctx.enter_context(tc.tile_pool(name="vp", bufs=16))
    es_pool = ctx.enter_context(tc.tile_pool(name="ep", bufs=16))
    vw_pool = ctx.enter_context(tc.tile_pool(name="vwp", bufs=4))
    rc_pool = ctx.enter_context(tc.tile_pool(name="rcp", bufs=2))
    os_pool = ctx.enter_context(tc.tile_pool(name="osp", bufs=6))
    # PSUM budget (8 banks): psn ring 2x[128,1024]=4, psd 1, pw 1, poh 2
    stage = ctx.enter_context(tc.tile_pool(name="stage", bufs=2, space="PSUM"))
    pvp = ctx.enter_context(tc.tile_pool(name="pvp", bufs=2, space="PSUM"))

    # ---- constants ----
    ones16 = const.tile([128, 128], F16)
    nc.gpsimd.memset(ones16, 1.0)

    # ---- weights (first matmul needs only wqk8 + x80: issue them first) ----
    wqk8_sb = w8_pool.tile([128, 4, 2, FQK], F8, name="wqk8", tag="wqk8")
    for i in range(2):
        nc.sync.dma_start(wqk8_sb[:, :, i, :], wqk8_d.ap().rearrange(
            "(dcp p) (two f) -> p dcp two f", p=128, two=2)[:, :, i, :])
    x80 = x8_pool.tile([128, 4, 2, QB], F8, name="x8", tag="x8")
    for i in range(2):
        nc.sync.dma_start(x80[:, :, i, :], x8_d.ap().rearrange(
            "(dcp p) (two s) -> p dcp two s", p=128, two=2)[:, :, i, 0:QB])
    bqk_sb = const.tile([128, 4], F32)
    nc.sync.dma_start(bqk_sb, bqk_d.ap().rearrange("(f p) -> p f", p=128))
    bvb_sb = const.tile([128, FV], F32)
    nc.sync.dma_start(bvb_sb, bvb_d.ap())
    xT0 = xt_pool.tile([128, 8, QB], F16, name="xt", tag="xt")
    nc.sync.dma_start(xT0, x_d.ap().rearrange(
        "(dc p) s -> p dc s", p=128)[:, :, 0:QB])
    wv_sb = wvp.tile([128, 8, FV], F16, name="wv", tag="wv")
    nc.sync.dma_start(wv_sb, wv_d.ap().rearrange("(dc p) f -> p dc f", p=128))
    wo_sb = wop.tile([128, 2, D], F16, name="wo", tag="wo")
    nc.sync.dma_start(wo_sb, wo_d.ap().rearrange("(c p) f -> p c f", p=128))
    # ---- phase B: QKV projection groups ----
    qkT = [qk_pool.tile([128, S], F8, name=f"qkT{f}", tag=f"qkT{f}", bufs=1)
           for f in range(4)]
    v_tiles = [None] * (NSC * 4)

    def b_groups(sc, tag, tag_v=None):
        """Closures for chunk sc: 4 fp8 QK feature groups + 4 fp16 V seq
        groups, each through one `tag` (QK) / `tag_v` (V) psum slot."""
        tag_v = tag if tag_v is None else tag_v
        tiles = {}

        def load_x():
            if sc == 0:
                tiles["xT"], tiles["x8"] = xT0, x80
                return
            x8 = x8_pool.tile([128, 4, 2, QB], F8, name="x8", tag="x8")
            for i in range(2):
                nc.sync.dma_start(x8[:, :, i, :], x8_d.ap().rearrange(
                    "(dcp p) (two s) -> p dcp two s", p=128, two=2)
                    [:, :, i, sc * QB:(sc + 1) * QB])
            xt = xt_pool.tile([128, 8, QB], F16, name="xt", tag="xt")
            nc.sync.dma_start(xt, x_d.ap().rearrange(
                "(dc p) s -> p dc s", p=128)[:, :, sc * QB:(sc + 1) * QB])
            tiles["xT"], tiles["x8"] = xt, x8

        def qk_group(f):
            def emit():
                pq = stage.tile([128, QB], F32, name="pq", tag=tag,
                                bufs=(2 if tag == "stage" else 1))
                for dcp in range(4):
                    nc.tensor.matmul(
                        pq, wqk8_sb[:, dcp, :, f * 128:(f + 1) * 128],
                        tiles["x8"][:, dcp, :, :], start=(dcp == 0),
                        stop=(dcp == 3), perf_mode=DR)
                nc.vector.tensor_scalar(
                    qkT[f][:, sc * QB:(sc + 1) * QB], pq,
                    1.0 / W8SCALE, bqk_sb[:, f:f + 1],
                    op0=mybir.AluOpType.mult, op1=mybir.AluOpType.add)
            return emit

        def v_group(sb):
            def emit():
                pv = stage.tile([128, FV], F32, name="pvq", tag=tag_v,
                                bufs=(2 if tag_v == "stage" else 1))
                for dc in range(8):
                    nc.tensor.matmul(
                        pv, tiles["xT"][:, dc, sb * 128:(sb + 1) * 128],
                        wv_sb[:, dc, :], start=(dc == 0), stop=(dc == 7))
                vt = v_pool.tile([128, NH, HD], F16, name="vt", tag="vt")
                nc.vector.tensor_add(
                    vt, pv.rearrange("p (h e) -> p h e", h=NH),
                    bvb_sb.rearrange("p (h e) -> p h e", h=NH))
                v_tiles[sc * 4 + sb] = vt
            return emit

        def first():
            load_x()
            qk_group(0)()
        qk_groups = [first] + [qk_group(f) for f in range(1, 4)]
        return qk_groups, [v_group(sb) for sb in range(4)]

    # ---- phase C ----
    def koff(qi, kb):
        return max(0, (kb - qi * 4)) * KB

    filler = []

    def pop_filler(n=1):
        for _ in range(n):
            if filler:
                filler.pop(0)()

    def emit_unit(qi, base, h, poh, nkb, do_pv=True):
        """Scores + exp (+ PV) for one (head, kb-pair). Returns es."""
        p, e = h // 2, h % 2
        r0 = e * 64
        off0 = koff(qi, base)
        psn = stage.tile([128, 2 * QB], F32, name="psn", tag="stage")
        for j2 in range(2):
            kb = base + j2
            off = koff(qi, kb)
            nc.tensor.matmul(
                psn[:, j2 * QB + off:(j2 + 1) * QB],
                qkT[2 + p][r0:r0 + 64, kb * KB:(kb + 1) * KB],
                qkT[p][r0:r0 + 64, qi * QB + off:(qi + 1) * QB],
                start=True, stop=True, skip_group_check=True)
        es = es_pool.tile([128, 2 * QB], F16, name="es", tag="es")
        nc.scalar.activation(es[:, off0:], psn[:, off0:], EXP, scale=SCALE)
        for j2 in range(2):
            kb = base + j2
            j = kb - qi * 4
            if j >= 0:
                off = j * KB
                nc.gpsimd.affine_select(
                    out=es[:, j2 * QB + off:(j2 + 1) * QB],
                    in_=es[:, j2 * QB + off:(j2 + 1) * QB],
                    compare_op=mybir.AluOpType.is_ge,
                    fill=0.0, base=0,
                    pattern=[[1, QB - off]],
                    channel_multiplier=-1,
                )
        if do_pv:
            emit_pv(qi, base, h, poh, nkb, es)
        return es

    def emit_pv(qi, base, h, poh, nkb, es):
        p, e = h // 2, h % 2
        r0 = e * 64
        for j2 in range(2):
            kb = base + j2
            off = koff(qi, kb)
            nc.tensor.matmul(
                poh[p][r0:r0 + 64, off:QB],
                v_tiles[kb][:, h, :],
                es[:, j2 * QB + off:(j2 + 1) * QB],
                start=(kb == 0), stop=(kb == nkb - 1),
                skip_group_check=True)

    def emit_denoms(qi, base, nkb, psd, es4):
        for j2 in range(2):
            kb = base + j2
            off = koff(qi, kb)
            for h in range(4):
                nc.tensor.matmul(
                    psd[32 * h:32 * h + 1, off:QB],
                    ones16[:, 0:1],
                    es4[h][:, j2 * QB + off:(j2 + 1) * QB],
                    start=(kb == 0), stop=(kb == nkb - 1),
                    tile_position=(0, 32 * h),
                    skip_group_check=True)

    def make_denorm_b(qi, rc16, poh):
        """Closure: broadcast 1/denom + normalize poh into vwT, then queue
        the Wo output groups."""
        def run():
            vwT = []
            for p in range(2):
                pb2 = stage.tile([128, 2 * QB], F32, name="pb2", tag="stage")
                for e in range(2):
                    h = 2 * p + e
                    nc.tensor.matmul(pb2[:, e * QB:(e + 1) * QB],
                                     ones16[32 * h:32 * h + 1, :],
                                     rc16[32 * h:32 * h + 1, :],
                                     start=True, stop=True,
                                     tile_position=(32 * h, 0),
                                     skip_group_check=True)
                bcs = rc_pool.tile([128, QB], F32, name="bcs", tag="bcs")
                for e in range(2):
                    r0 = e * 64
                    nc.vector.tensor_copy(
                        bcs[r0:r0 + 64, :],
                        pb2[r0:r0 + 64, e * QB:(e + 1) * QB])
                vw = vw_pool.tile([128, QB], F16, name=f"vwT{p}", tag="vw")
                nc.vector.tensor_mul(vw, poh[p], bcs)
                vwT.append(vw)
            last = qi == NSC - 1
            filler.extend(
                wo_group(qi, vwT, ql, do, tag=("stage" if last else "pw"))
                for ql in range(4) for do in range(2))
        return run

    def wo_group(qi, vwT, ql, do, tag="pw"):
        def emit():
            pw = stage.tile([128, QB], F32, name="pw", tag=tag,
                            bufs=(1 if tag != "stage" else 2))
            for c in range(2):
                nc.tensor.matmul(pw, vwT[c][:, ql * 128:(ql + 1) * 128],
                                 wo_sb[:, c, do * QB:(do + 1) * QB],
                                 start=(c == 0), stop=(c == 1))
            osb = os_pool.tile([128, QB], F16, name="osb", tag="osb")
            nc.vector.tensor_copy(osb, pw)
            nc.sync.dma_start(
                out_d.ap()[qi * QB + ql * 128: qi * QB + (ql + 1) * 128,
                           do * QB:(do + 1) * QB], osb)
        return emit

    def emit_C(qi, denorm_b_prev):
        """Full attention for query chunk qi. Returns this qi's denorm_b."""
        nkb = (qi + 1) * 4
        poh = None
        es0 = []
        psd = stage.tile([128, QB], F32, name="psd", tag="psd", bufs=1)
        for base in range(0, nkb, 2):
            es4 = []
            for h in range(4):
                es = emit_unit(qi, base, h, poh, nkb, do_pv=(base > 0))
                es4.append(es)
                if base > 0:
                    pop_filler()
            emit_denoms(qi, base, nkb, psd, es4)
            if base == 0:
                # qi-boundary catch-up while this qi's first exps drain:
                # previous qi's normalization (frees its poh slots), then
                # this qi's deferred base-0 PV matmuls.
                if denorm_b_prev is not None:
                    denorm_b_prev()
                pop_filler(4 if qi == 0 else 2)
                poh = [pvp.tile([128, QB], F32, name=f"poh{p}", tag="pv")
                       for p in range(2)]
                for h in range(4):
                    emit_pv(qi, base, h, poh, nkb, es4[h])
        # reciprocal chain (DVE only; PE stream continues into next qi)
        dsb = rc_pool.tile([128, QB], F32, name="dsb", tag="dsb")
        nc.vector.tensor_copy(dsb, psd)
        rcf = rc_pool.tile([128, QB], F32, name="rcf", tag="rcf")
        nc.vector.reciprocal_approx_fast(rcf, dsb)
        rc16 = rc_pool.tile([128, QB], F16, name="rc16", tag="rc16")
        nc.vector.tensor_copy(rc16, rcf)
        return make_denorm_b(qi, rc16, poh)

    # ---- schedule ----
    qk0, v0 = b_groups(0, "stage", "pw")
    for g in qk0:
        g()
    filler.extend(v0)
    denorm_b = None
    for qi in range(NSC):
        if qi in (0, 1):
            # B1 drains during C0, B2 during C1
            qk, vg = b_groups(qi + 1, "pw")
            filler.extend(qk + vg)
        elif qi == NSC - 1:
            # B3: C3's scores need chunk-3 Q/K up front, so the QK groups
            # emit directly (they fill C2's ScalarE-bound tail); the V
            # groups (only needed from kb block 12) drain as C3 fillers.
            qk, vg = b_groups(qi, "pw")
            for g in qk:
                g()
            filler.extend(vg)
        denorm_b = emit_C(qi, denorm_b)
    denorm_b()
    while filler:
        filler.pop(0)()


_COMPILED = None


def get_compiled():
    global _COMPILED
    if _COMPILED is not None:
        return _COMPILED
    nc = bacc.Bacc("TRN2", target_bir_lowering=False, debug=False,
                   enable_asserts=False, num_devices=NCORES)
    x_d = nc.dram_tensor("x", [D, S], F16, kind="ExternalInput")
    x8_d = nc.dram_tensor("x8", [D // 2, 2 * S], F8, kind="ExternalInput")
    wqk8_d = nc.dram_tensor("wqk8", [D // 2, 2 * FQK], F8,
                            kind="ExternalInput")
    wv_d = nc.dram_tensor("wv", [D, FV], F16, kind="ExternalInput")
    bqk_d = nc.dram_tensor("bqk", [FQK], F32, kind="ExternalInput")
    bvb_d = nc.dram_tensor("bvb", [128, FV], F32, kind="ExternalInput")
    wo_d = nc.dram_tensor("wo", [FV, D], F16, kind="ExternalInput")
    out_d = nc.dram_tensor("out", [S, D], F16, kind="ExternalOutput")
    with tile.TileContext(nc) as tc:
        with ExitStack() as ctx:
            _build_body(ctx, tc, x_d, x8_d, wqk8_d, wv_d, bqk_d, bvb_d, wo_d,
                        out_d)
    nc.compile()
    _COMPILED = nc
    return nc


def _pack_pairs(a):
    """[256*k, n] -> pair-packed [128*k, 2*n] for DoubleRow operands."""
    k = a.shape[0] // 256
    return np.ascontiguousarray(
        a.reshape(k, 2, 128, a.shape[1]).transpose(0, 2, 1, 3)
        .reshape(k * 128, 2 * a.shape[1]))


def make_in_maps(x, Wqkv, bqkv, Wo):
    f8 = ml_dtypes.float8_e4m3fn
    x = np.ascontiguousarray(np.asarray(x, dtype=np.float32))
    Wqkv = np.asarray(Wqkv, dtype=np.float32)
    bqkv = np.asarray(bqkv, dtype=np.float32)
    Wo = np.asarray(Wo, dtype=np.float32)
    in_maps = []
    for c in range(NCORES):
        b, hg = divmod(c, NHG)
        qs = slice(hg * FV, (hg + 1) * FV)
        ks = slice(D + hg * FV, D + (hg + 1) * FV)
        vs = slice(2 * D + hg * FV, 2 * D + (hg + 1) * FV)
        xT = np.ascontiguousarray(x[b].T)  # [D, S] f32
        wqk = np.concatenate([Wqkv[:, qs], Wqkv[:, ks]], axis=1)  # [D, FQK]
        in_maps.append({
            "x": xT.astype(np.float16),
            "x8": _pack_pairs(xT).astype(f8),
            "wqk8": _pack_pairs(wqk * W8SCALE).astype(f8),
            "wv": np.ascontiguousarray(Wqkv[:, vs]).astype(np.float16),
            "bqk": np.ascontiguousarray(
                np.concatenate([bqkv[qs], bqkv[ks]])),
            "bvb": np.ascontiguousarray(
                np.broadcast_to(bqkv[vs], (128, FV)).copy()),
            "wo": np.ascontiguousarray(Wo[hg * FV:(hg + 1) * FV, :]).astype(np.float16),
        })
    return in_maps


def run_sharded(x, Wqkv, bqkv, Wo, bo, **spmd_kwargs):
    nc = get_compiled()
    in_maps = make_in_maps(x, Wqkv, bqkv, Wo)
    res = run_bass_kernel_spmd(nc, in_maps, core_ids=list(range(NCORES)),
                               **spmd_kwargs)
    out = np.zeros((B, S, D), np.float32)
    for c in range(NCORES):
        out[c // NHG] += res.results[c]["out"].astype(np.float32)
    out += np.asarray(bo, dtype=np.float32)
    return out, res


def kernel(x, mask, Wqkv, bqkv, Wo, bo):
    out, _ = run_sharded(x, Wqkv, bqkv, Wo, bo)
    return out
